# revision 23
# baseline (speedup 1.0000x reference)
"""Trainium2 Bass kernel for causal self-attention (B=4, T=2048, C=2048, H=16).

Sharding: 8 cores = DP4 (batch) x TP2 (8 heads each). Each core:
  P1  qk' = (x @ Wqk)^T computed in [j, t] layout, kept resident in SBUF
  P2  v   = x @ Wv in natural [t, j] layout (stationary = xT slab blocks)
  P3  per head: s_T = k'^T q' -> exp -> causal mask -> av + ones-den
      -> y'_h = o' * recip(den); y'_h staged to DRAM; every 2 heads an
      AllGather over the core pair exchanges y so both cores hold all 16
      global heads' y (overlapped with remaining attention compute).
  P4  out[t, n_half] = sum over all 16 global heads y'^T @ Wp + bias,
      written directly to the external output (each core owns half the
      output columns; no post-hoc reduction needed). y blocks streamed
      back from the AllGather DRAM buffer.

All matmuls bf16 with fp32 PSUM accumulation; softmax in fp32 on ACT/DVE.
Host side: shard/cast/transpose inputs, concat output column halves.
"""
import math
import numpy as np
import ml_dtypes

import concourse.bass as bass
import concourse.bacc as bacc
import concourse.mybir as mybir
import concourse.tile as tile

F32 = mybir.dt.float32
BF16 = mybir.dt.bfloat16
AF = mybir.ActivationFunctionType

D = 128          # head dim (fixed: partition size)
N_CORES = 8
PAIRS = [[0, 1], [2, 3], [4, 5], [6, 7]]
AG_CHUNK = 2     # heads per AllGather


class Cfg:
    def __init__(self, T=2048, H_TOT=16, HPC=8, B=4):
        self.T = T                    # sequence length
        self.H_TOT = H_TOT            # total heads
        self.HPC = HPC                # heads per core
        self.B = B
        self.C = H_TOT * D            # model dim
        self.CP = HPC * D             # per-core head cols
        self.CH = self.C // 2         # per-core output cols
        self.TCH = 512                # ti chunk width
        assert T % self.TCH == 0 and T % D == 0


def build_kernel(cfg: Cfg):
    T, C, CP, HPC, TCH, CH = cfg.T, cfg.C, cfg.CP, cfg.HPC, cfg.TCH, cfg.CH
    NC_CH = C // D                # c-chunks (contraction)
    NJB = 2 * HPC                 # qk' j-blocks (q heads then k heads)
    NTB = T // D                  # t-blocks
    NIC = T // TCH                # ti chunks
    NTR = T // 512                # t-ranges (slab columns)
    NPR = CH // 512               # proj n-ranges
    NAG = HPC // AG_CHUNK         # AllGather chunks
    NH_G = 2 * HPC                # global heads in P4 contraction
    scale = 1.0 / math.sqrt(D)

    nc = bacc.Bacc()
    xT = nc.declare_dram_parameter("xT", [C, T], BF16, isOutput=False)
    wqk = nc.declare_dram_parameter("wqk", [C, 2 * CP], BF16, isOutput=False)
    wv = nc.declare_dram_parameter("wv", [C, CP], BF16, isOutput=False)
    wp = nc.declare_dram_parameter("wp", [C, CH], BF16, isOutput=False)
    bqk = nc.declare_dram_parameter("bqk", [D, NJB], F32, isOutput=False)
    bv = nc.declare_dram_parameter("bv", [1, CP], BF16, isOutput=False)
    bp = nc.declare_dram_parameter("bp", [1, CH], BF16, isOutput=False)
    masks = nc.declare_dram_parameter("masks", [D, 4 * TCH], BF16, isOutput=False)
    out_ext = nc.declare_dram_parameter("out", [T, CH], F32, isOutput=True)

    ag_in = nc.dram_tensor("ag_in", [HPC, D, T], BF16)
    ag_out = nc.dram_tensor("ag_out", [NAG, 2, AG_CHUNK, D, T], BF16)

    with tile.TileContext(nc) as tc:
        with tc.tile_pool(name="const", bufs=1) as constp:
            bqk_t = constp.tile([D, NJB], F32, name="bqk_t")
            nc.sync.dma_start(bqk_t[:], bqk[:, :])
            bv_t = constp.tile([1, CP], BF16, name="bv_t")
            nc.sync.dma_start(bv_t[:], bv[:, :])
            bp_t = constp.tile([1, CH], BF16, name="bp_t")
            nc.sync.dma_start(bp_t[:], bp[:, :])
            mask_sb = constp.tile([D, 4 * TCH], BF16, name="mask_sb")
            nc.sync.dma_start(mask_sb[:], masks[:, :])
            ones_sq = constp.tile([D, D], BF16, name="ones_sq")
            nc.vector.memset(ones_sq[:], 1.0)
            ones_row = constp.tile([1, D], BF16, name="ones_row")
            nc.vector.memset(ones_row[:], 1.0)

            # broadcast bias rows -> [D, .] tiles (ones-matmul, one time)
            bv_bc = constp.tile([D, CP], BF16, name="bv_bc")
            bp_bc = constp.tile([D, CH], BF16, name="bp_bc")
            with tc.tile_pool(name="pbias", bufs=4, space="PSUM") as pbias:
                for vr in range(CP // 512):
                    psb = pbias.tile([D, 512], F32, name="psbv", tag="pb")
                    nc.tensor.matmul(psb[:], ones_row[:],
                                     bv_t[:, vr * 512:(vr + 1) * 512],
                                     start=True, stop=True)
                    nc.vector.tensor_copy(bv_bc[:, vr * 512:(vr + 1) * 512],
                                          psb[:])
                for nr in range(NPR):
                    psb = pbias.tile([D, 512], F32, name="psbp", tag="pb")
                    nc.tensor.matmul(psb[:], ones_row[:],
                                     bp_t[:, nr * 512:(nr + 1) * 512],
                                     start=True, stop=True)
                    nc.vector.tensor_copy(bp_bc[:, nr * 512:(nr + 1) * 512],
                                          psb[:])

            # manual-lifetime pools (reserve-at-push, LIFO release):
            # qkp/vres live to P3 end, xtp to P2 end, wpp pushed after xtp
            # pops and lives to P4 end.
            qkp_ctx = tc.tile_pool(name="qkp", bufs=1)
            qkp = qkp_ctx.__enter__()
            vres_ctx = tc.tile_pool(name="vres", bufs=1)
            vres = vres_ctx.__enter__()
            xtp_ctx = tc.tile_pool(name="xtp", bufs=1)
            xtp = xtp_ctx.__enter__()
            xt = [[None] * NTR for _ in range(NC_CH)]

            # resident qk' tiles, [D, T] per j-block; freed after P3
            qks = [qkp.tile([D, T], BF16, name=f"qks{jb}") for jb in range(NJB)]

            # ---------------- P1: qk' ----------------
            with (
                tc.tile_pool(name="wqkp", bufs=32) as wqkp,
                tc.tile_pool(name="pq", bufs=8, space="PSUM") as pqp,
            ):
                # quad 0 weights interleaved with x slabs so jb0 starts fast
                wq_cur = []
                for c in range(NC_CH):
                    wt = wqkp.tile([D, 512], BF16, name="wq", tag="wq")
                    nc.sync.dma_start(wt[:], wqk[c * D:(c + 1) * D, 0:512])
                    wq_cur.append(wt)
                    for tr in range(NTR):
                        st = xtp.tile([D, 512], BF16, name=f"xt{c}_{tr}")
                        nc.sync.dma_start(
                            st[:], xT[c * D:(c + 1) * D, tr * 512:(tr + 1) * 512])
                        xt[c][tr] = st
                for jb in range(NJB):
                    if jb % 4 == 0 and jb > 0:
                        wq_cur = []
                        for c in range(NC_CH):
                            wt = wqkp.tile([D, 512], BF16, name="wq", tag="wq")
                            nc.sync.dma_start(
                                wt[:], wqk[c * D:(c + 1) * D,
                                           jb * D:(jb + 4) * D])
                            wq_cur.append(wt)
                    jo = (jb % 4) * D
                    ps = [pqp.tile([D, 512], F32, name="pq", tag="pq")
                          for _ in range(NTR)]
                    for c in range(NC_CH):
                        for tr in range(NTR):
                            nc.tensor.matmul(
                                ps[tr][:], wq_cur[c][:, jo:jo + D],
                                xt[c][tr][:],
                                start=(c == 0), stop=(c == NC_CH - 1))
                    for tr in range(NTR):
                        nc.vector.tensor_scalar_add(
                            qks[jb][:, tr * 512:(tr + 1) * 512], ps[tr][:],
                            bqk_t[:, jb:jb + 1])

            # ---------------- P2: v ----------------
            v_sb = [vres.tile([D, CP], BF16, name=f"v{tb}")
                    for tb in range(NTB)]
            with (
                tc.tile_pool(name="wvp", bufs=1) as wvp,
                tc.tile_pool(name="pv", bufs=4, space="PSUM") as pvp,
            ):
                wv_t = []
                for c in range(NC_CH):
                    t = wvp.tile([D, CP], BF16, name=f"wv{c}")
                    nc.sync.dma_start(t[:], wv[c * D:(c + 1) * D, :])
                    wv_t.append(t)
                NVR = CP // 512
                for tb in range(NTB):
                    ps = [pvp.tile([D, 512], F32, name="pv", tag="pv")
                          for _ in range(NVR)]
                    for c in range(NC_CH):
                        blk = xt[c][tb // 4][:, (tb % 4) * D:(tb % 4 + 1) * D]
                        for vr in range(NVR):
                            nc.tensor.matmul(
                                ps[vr][:], blk,
                                wv_t[c][:, vr * 512:(vr + 1) * 512],
                                start=(c == 0), stop=(c == NC_CH - 1))
                    for vr in range(NVR):
                        nc.vector.tensor_add(
                            v_sb[tb][:, vr * 512:(vr + 1) * 512], ps[vr][:],
                            bv_bc[:, vr * 512:(vr + 1) * 512])
            xtp_ctx.__exit__(None, None, None)

            # P4 weights: preload during P3 (rows already in arrival order)
            wpp_ctx = tc.tile_pool(name="wpp", bufs=1)
            wpp = wpp_ctx.__enter__()
            wp_t = []
            for k in range(NH_G):
                t = wpp.tile([D, CH], BF16, name=f"wp{k}")
                nc.sync.dma_start(t[:], wp[k * D:(k + 1) * D, :])
                wp_t.append(t)

            # ---------------- P3: attention + y exchange ----------------
            GSZ = 4
            ygs_ctx = tc.tile_pool(name="ygs", bufs=32)
            ygs = ygs_ctx.__enter__()
            yg_pre = [None] * NH_G
            with (
                tc.tile_pool(name="ylocal", bufs=2) as ylp,
                tc.tile_pool(name="attp", bufs=4) as attp,
                tc.tile_pool(name="ps_s", bufs=2, space="PSUM") as ps_s,
                tc.tile_pool(name="ps_o", bufs=2, space="PSUM") as ps_o,
                tc.tile_pool(name="ps_d", bufs=2, space="PSUM") as ps_d,
                tc.tile_pool(name="normp", bufs=4) as normp,
            ):
                # one flat software pipeline over all (head, chunk, pair):
                # S/exp/mask run 2 pairs ahead of AV, across all boundaries.
                # Chunks processed largest-first so lookahead pairs are
                # rarely masked (mask-mul stays off the critical path).
                pairs = []
                for h in range(HPC):
                    for ic in range(NIC - 1, -1, -1):
                        npair = ((ic + 1) * TCH // D) // 2
                        for pr in range(npair):
                            pairs.append((h, ic, pr, npair))
                NPAIRS = len(pairs)
                yt_t = {}
                po_t = {}
                atts = {}

                def emit_s(i):
                    h, ic, pr, npair = pairs[i]
                    qp = qks[2 * h]
                    kp = qks[2 * h + 1]
                    ti0 = ic * TCH
                    s_p = ps_s.tile([D, 2 * TCH], F32, name="sp", tag="sp")
                    for hf in range(2):
                        tk = 2 * pr + hf
                        nc.tensor.matmul(
                            s_p[:, hf * TCH:(hf + 1) * TCH],
                            kp[:, tk * D:(tk + 1) * D],
                            qp[:, ti0:ti0 + TCH], start=True, stop=True)
                    att = attp.tile([D, 2 * TCH], BF16, name="att", tag="att")
                    nc.scalar.activation(att[:], s_p[:], AF.Exp,
                                         bias=0.0, scale=scale)
                    # causal mask: multiply by 0/1 (bf16 SBUF, 2x DVE)
                    pdiag = pr - (npair - 2)
                    if pdiag >= 0:
                        nc.vector.tensor_mul(
                            att[:], att[:],
                            mask_sb[:, pdiag * 2 * TCH:(pdiag + 1) * 2 * TCH])
                    atts[i] = att

                def emit_av(i):
                    h, ic, pr, npair = pairs[i]
                    ntk = 2 * npair
                    ti0 = ic * TCH
                    if pr == 0:
                        po_t[(h, ic)] = (
                            ps_o.tile([D, TCH], F32, name="po", tag="po"),
                            ps_d.tile([D, TCH], F32, name="pd", tag="pd"))
                    po, pd = po_t[(h, ic)]
                    att = atts.pop(i)
                    for hf in range(2):
                        tk = 2 * pr + hf
                        a = att[:, hf * TCH:(hf + 1) * TCH]
                        nc.tensor.matmul(
                            po[:], v_sb[tk][:, h * D:(h + 1) * D], a,
                            start=(tk == 0), stop=(tk == ntk - 1))
                        nc.tensor.matmul(
                            pd[:], ones_sq[:], a,
                            start=(tk == 0), stop=(tk == ntk - 1))
                    if pr == npair - 1:
                        if h not in yt_t:
                            yt_t[h] = ylp.tile([D, T], BF16, name="yt",
                                               tag="yt")
                        yt = yt_t[h]
                        rec = normp.tile([D, TCH], F32, name="rec", tag="rec")
                        nc.vector.reciprocal(rec[:], pd[:])
                        nc.vector.tensor_mul(yt[:, ti0:ti0 + TCH], po[:],
                                             rec[:])
                        del po_t[(h, ic)]
                        if ic == 0:
                            finish_head(h)

                def finish_head(h):
                    nc.sync.dma_start(ag_in[h, :, :], yt_t[h][:])
                    if h % AG_CHUNK == AG_CHUNK - 1:
                        cchunk = h // AG_CHUNK
                        nc.gpsimd.collective_compute(
                            "AllGather",
                            mybir.AluOpType.bypass,
                            ins=[ag_in[cchunk * AG_CHUNK:
                                       (cchunk + 1) * AG_CHUNK, :, :]],
                            outs=[ag_out[cchunk, :, :, :, :]],
                            replica_groups=PAIRS,
                        )
                        # prefetch P4 group-0 y blocks for this chunk
                        for kk in range(4 * cchunk, 4 * cchunk + 4):
                            yg = ygs.tile([D, GSZ * D], BF16, name="yg",
                                          tag="yg")
                            nc.sync.dma_start(
                                yg[:], ag_out[kk // 4, (kk % 4) // 2, kk % 2,
                                              :, 0:GSZ * D])
                            yg_pre[kk] = yg

                emit_s(0)
                emit_s(1)
                for i in range(NPAIRS):
                    if i + 2 < NPAIRS:
                        emit_s(i + 2)
                    emit_av(i)
            # ---------------- P4: projection (all 16 global heads) -------
            # arrival order k -> ag_out[chunk=k//4, contrib=(k%4)//2, k%2]
            with (
                tc.tile_pool(name="pp", bufs=8, space="PSUM") as ppp,
                tc.tile_pool(name="post", bufs=3) as post,
            ):
                for g0 in range(0, NTB, GSZ):
                    if g0 == 0:
                        ygt = yg_pre
                    else:
                        ygt = []
                        for k in range(NH_G):
                            yg = ygs.tile([D, GSZ * D], BF16, name="yg",
                                          tag="yg")
                            nc.sync.dma_start(
                                yg[:], ag_out[k // 4, (k % 4) // 2, k % 2, :,
                                              g0 * D:(g0 + GSZ) * D])
                            ygt.append(yg)
                    pss = [[ppp.tile([D, 512], F32, name="pp", tag="pp")
                            for _ in range(NPR)] for _ in range(GSZ)]
                    for k in range(NH_G):
                        for ti in range(GSZ):
                            blk = ygt[k][:, ti * D:(ti + 1) * D]
                            for nr in range(NPR):
                                nc.tensor.matmul(
                                    pss[ti][nr][:], blk,
                                    wp_t[k][:, nr * 512:(nr + 1) * 512],
                                    start=(k == 0), stop=(k == NH_G - 1))
                    for ti in range(GSZ):
                        tb = g0 + ti
                        st = post.tile([D, CH], F32, name="pst", tag="pst")
                        for nr in range(NPR):
                            nc.vector.tensor_add(
                                st[:, nr * 512:(nr + 1) * 512],
                                pss[ti][nr][:],
                                bp_bc[:, nr * 512:(nr + 1) * 512])
                        nc.sync.dma_start(
                            out_ext[tb * D:(tb + 1) * D, :], st[:])
            ygs_ctx.__exit__(None, None, None)
            wpp_ctx.__exit__(None, None, None)
            vres_ctx.__exit__(None, None, None)
            qkp_ctx.__exit__(None, None, None)
    nc.finalize()
    return nc


def _prep_inputs(cfg: Cfg, x, w_attn, b_attn, w_proj, b_proj):
    """Host-side shard/cast. Returns in_maps (list of dicts per core)."""
    T, C, CP, HPC, CH = cfg.T, cfg.C, cfg.CP, cfg.HPC, cfg.CH
    bf = ml_dtypes.bfloat16
    wq = w_attn[:, 0:C]
    wk = w_attn[:, C:2 * C]
    wvf = w_attn[:, 2 * C:3 * C]
    bq, bk, bvf = b_attn[0:C], b_attn[C:2 * C], b_attn[2 * C:3 * C]

    masks = np.zeros((D, 4 * cfg.TCH), dtype=bf)
    f = np.arange(cfg.TCH)[None, :]
    p = np.arange(D)[:, None]
    for k in range(4):
        keep = (f - p >= 128 * k)
        masks[:, k * cfg.TCH:(k + 1) * cfg.TCH] = np.where(
            keep, 1.0, 0.0).astype(bf)

    # P4 contraction (arrival) order: for AG chunk c, contributor m, head j
    # -> global head 8*m + 2*c + j
    arrival = [8 * m + AG_CHUNK * c + j
               for c in range(HPC // AG_CHUNK)
               for m in range(2)
               for j in range(AG_CHUNK)]

    in_maps = []
    for core in range(N_CORES):
        b = core // 2
        g = core % 2
        h0 = g * HPC * D            # first col of this head group
        sl = slice(h0, h0 + CP)
        xTc = np.ascontiguousarray(x[b].T).astype(bf)
        wqk_cols = []
        for h in range(HPC):
            hs = slice(h0 + h * D, h0 + (h + 1) * D)
            wqk_cols.append(wq[:, hs])
            wqk_cols.append(wk[:, hs])
        wqk_c = np.concatenate(wqk_cols, axis=1).astype(bf)
        wv_c = wvf[:, sl].astype(bf)
        # full-row proj weights, rows in arrival order, columns = core half
        wp_rows = [w_proj[gh * D:(gh + 1) * D, g * CH:(g + 1) * CH]
                   for gh in arrival]
        wp_c = np.concatenate(wp_rows, axis=0).astype(bf)
        bqk_cols = []
        for h in range(HPC):
            hs = slice(h0 + h * D, h0 + (h + 1) * D)
            bqk_cols.append(bq[hs])
            bqk_cols.append(bk[hs])
        bqk_c = np.ascontiguousarray(np.stack(bqk_cols, axis=1)).astype(np.float32)
        in_maps.append({
            "xT": xTc,
            "wqk": wqk_c,
            "wv": wv_c,
            "wp": wp_c,
            "bqk": bqk_c,
            "bv": bvf[sl].reshape(1, CP).astype(bf),
            "bp": b_proj[g * CH:(g + 1) * CH].reshape(1, CH).astype(bf),
            "masks": masks,
        })
    return in_maps


_CFG = Cfg()


def kernel(x, w_attn, b_attn, w_proj, b_proj, _trace=False, _cfg=None):
    from concourse.bass_utils import run_bass_kernel_spmd
    cfg = _cfg or _CFG
    x = np.asarray(x, dtype=np.float32)
    w_attn = np.asarray(w_attn, dtype=np.float32)
    b_attn = np.asarray(b_attn, dtype=np.float32)
    w_proj = np.asarray(w_proj, dtype=np.float32)
    b_proj = np.asarray(b_proj, dtype=np.float32)

    in_maps = _prep_inputs(cfg, x, w_attn, b_attn, w_proj, b_proj)
    nc = build_kernel(cfg)
    res = run_bass_kernel_spmd(nc, in_maps, list(range(N_CORES)), trace=_trace)
    outs = []
    for b in range(cfg.B):
        left = res.results[2 * b]["out"]
        right = res.results[2 * b + 1]["out"]
        outs.append(np.concatenate([left, right], axis=1))
    full = np.stack(outs, axis=0).astype(np.float32)
    if _trace:
        kernel.last_exec_time_ns = res.exec_time_ns
        kernel.last_mean_exec_time_ns = res.mean_exec_time_ns
        kernel.last_scope_times = res.per_core_scope_times
    return full


# revision 24
# speedup vs baseline: 1.0837x; 1.0837x over previous
"""Trainium2 Bass kernel for causal self-attention (B=4, T=2048, C=2048, H=16).

Sharding: 8 cores = DP4 (batch) x TP2 (8 heads each). Each core:
  P1  qk' = (x @ Wqk)^T computed in [j, t] layout, kept resident in SBUF
  P2  v   = x @ Wv in natural [t, j] layout (stationary = xT slab blocks)
  P3  per head: s_T = k'^T q' -> exp -> causal mask -> av + ones-den
      -> y'_h = o' * recip(den); y'_h staged to DRAM; every 2 heads an
      AllGather over the core pair exchanges y so both cores hold all 16
      global heads' y (overlapped with remaining attention compute).
  P4  out[t, n_half] = sum over all 16 global heads y'^T @ Wp + bias,
      written directly to the external output (each core owns half the
      output columns; no post-hoc reduction needed). y blocks streamed
      back from the AllGather DRAM buffer.

All matmuls bf16 with fp32 PSUM accumulation; softmax in fp32 on ACT/DVE.
Host side: shard/cast/transpose inputs, concat output column halves.
"""
import math
import numpy as np
import ml_dtypes

import concourse.bass as bass
import concourse.bacc as bacc
import concourse.mybir as mybir
import concourse.tile as tile

F32 = mybir.dt.float32
BF16 = mybir.dt.bfloat16
AF = mybir.ActivationFunctionType

D = 128          # head dim (fixed: partition size)
N_CORES = 8
PAIRS = [[0, 1], [2, 3], [4, 5], [6, 7]]
AG_CHUNK = 2     # heads per AllGather


class Cfg:
    def __init__(self, T=2048, H_TOT=16, HPC=8, B=4):
        self.T = T                    # sequence length
        self.H_TOT = H_TOT            # total heads
        self.HPC = HPC                # heads per core
        self.B = B
        self.C = H_TOT * D            # model dim
        self.CP = HPC * D             # per-core head cols
        self.CH = self.C // 2         # per-core output cols
        self.TCH = 512                # ti chunk width
        assert T % self.TCH == 0 and T % D == 0


def build_kernel(cfg: Cfg):
    T, C, CP, HPC, TCH, CH = cfg.T, cfg.C, cfg.CP, cfg.HPC, cfg.TCH, cfg.CH
    NC_CH = C // D                # c-chunks (contraction)
    NJB = 2 * HPC                 # qk' j-blocks (q heads then k heads)
    NTB = T // D                  # t-blocks
    NIC = T // TCH                # ti chunks
    NTR = T // 512                # t-ranges (slab columns)
    NPR = CH // 512               # proj n-ranges
    NAG = HPC // AG_CHUNK         # AllGather chunks
    NH_G = 2 * HPC                # global heads in P4 contraction
    scale = 1.0 / math.sqrt(D)

    nc = bacc.Bacc()
    xT = nc.declare_dram_parameter("xT", [C, T], BF16, isOutput=False)
    wqk = nc.declare_dram_parameter("wqk", [C, 2 * CP], BF16, isOutput=False)
    wv = nc.declare_dram_parameter("wv", [C, CP], BF16, isOutput=False)
    wp = nc.declare_dram_parameter("wp", [C, CH], BF16, isOutput=False)
    bqk = nc.declare_dram_parameter("bqk", [D, NJB], F32, isOutput=False)
    bv = nc.declare_dram_parameter("bv", [1, CP], BF16, isOutput=False)
    bp = nc.declare_dram_parameter("bp", [1, CH], BF16, isOutput=False)
    masks = nc.declare_dram_parameter("masks", [D, 4 * TCH], BF16, isOutput=False)
    out_ext = nc.declare_dram_parameter("out", [T, CH], F32, isOutput=True)

    ag_in = nc.dram_tensor("ag_in", [HPC, D, T], BF16)
    ag_out = nc.dram_tensor("ag_out", [NAG, 2, AG_CHUNK, D, T], BF16)

    with tile.TileContext(nc) as tc:
        with tc.tile_pool(name="const", bufs=1) as constp:
            bqk_t = constp.tile([D, NJB], F32, name="bqk_t")
            nc.sync.dma_start(bqk_t[:], bqk[:, :])
            bv_t = constp.tile([1, CP], BF16, name="bv_t")
            nc.sync.dma_start(bv_t[:], bv[:, :])
            bp_t = constp.tile([1, CH], BF16, name="bp_t")
            nc.sync.dma_start(bp_t[:], bp[:, :])
            mask_sb = constp.tile([D, 4 * TCH], BF16, name="mask_sb")
            nc.sync.dma_start(mask_sb[:], masks[:, :])
            ones_sq = constp.tile([D, D], BF16, name="ones_sq")
            nc.vector.memset(ones_sq[:], 1.0)
            ones_row = constp.tile([1, D], BF16, name="ones_row")
            nc.vector.memset(ones_row[:], 1.0)

            # broadcast bias rows -> [D, .] tiles (ones-matmul, one time)
            bv_bc = constp.tile([D, CP], BF16, name="bv_bc")
            bp_bc = constp.tile([D, CH], BF16, name="bp_bc")
            with tc.tile_pool(name="pbias", bufs=4, space="PSUM") as pbias:
                for vr in range(CP // 512):
                    psb = pbias.tile([D, 512], F32, name="psbv", tag="pb")
                    nc.tensor.matmul(psb[:], ones_row[:],
                                     bv_t[:, vr * 512:(vr + 1) * 512],
                                     start=True, stop=True)
                    nc.vector.tensor_copy(bv_bc[:, vr * 512:(vr + 1) * 512],
                                          psb[:])
                for nr in range(NPR):
                    psb = pbias.tile([D, 512], F32, name="psbp", tag="pb")
                    nc.tensor.matmul(psb[:], ones_row[:],
                                     bp_t[:, nr * 512:(nr + 1) * 512],
                                     start=True, stop=True)
                    nc.vector.tensor_copy(bp_bc[:, nr * 512:(nr + 1) * 512],
                                          psb[:])

            # manual-lifetime pools (reserve-at-push, LIFO release):
            # qkp/vres live to P3 end, xtp to P2 end, wpp pushed after xtp
            # pops and lives to P4 end.
            qkp_ctx = tc.tile_pool(name="qkp", bufs=1)
            qkp = qkp_ctx.__enter__()
            vres_ctx = tc.tile_pool(name="vres", bufs=1)
            vres = vres_ctx.__enter__()
            xtp_ctx = tc.tile_pool(name="xtp", bufs=1)
            xtp = xtp_ctx.__enter__()
            xt = [[None] * NTR for _ in range(NC_CH)]

            # resident qk' tiles, [D, T] per j-block; freed after P3
            qks = [qkp.tile([D, T], BF16, name=f"qks{jb}") for jb in range(NJB)]

            # ---------------- P1: qk' ----------------
            with (
                tc.tile_pool(name="wqkp", bufs=32) as wqkp,
                tc.tile_pool(name="pq", bufs=8, space="PSUM") as pqp,
            ):
                # quad 0 weights interleaved with x slabs so jb0 starts fast
                wq_cur = []
                for c in range(NC_CH):
                    wt = wqkp.tile([D, 512], BF16, name="wq", tag="wq")
                    nc.sync.dma_start(wt[:], wqk[c * D:(c + 1) * D, 0:512])
                    wq_cur.append(wt)
                    for tr in range(NTR):
                        st = xtp.tile([D, 512], BF16, name=f"xt{c}_{tr}")
                        nc.sync.dma_start(
                            st[:], xT[c * D:(c + 1) * D, tr * 512:(tr + 1) * 512])
                        xt[c][tr] = st
                for jb in range(NJB):
                    if jb % 4 == 0 and jb > 0:
                        wq_cur = []
                        for c in range(NC_CH):
                            wt = wqkp.tile([D, 512], BF16, name="wq", tag="wq")
                            nc.sync.dma_start(
                                wt[:], wqk[c * D:(c + 1) * D,
                                           jb * D:(jb + 4) * D])
                            wq_cur.append(wt)
                    jo = (jb % 4) * D
                    ps = [pqp.tile([D, 512], F32, name="pq", tag="pq")
                          for _ in range(NTR)]
                    for c in range(NC_CH):
                        for tr in range(NTR):
                            nc.tensor.matmul(
                                ps[tr][:], wq_cur[c][:, jo:jo + D],
                                xt[c][tr][:],
                                start=(c == 0), stop=(c == NC_CH - 1))
                    for tr in range(NTR):
                        nc.vector.tensor_scalar_add(
                            qks[jb][:, tr * 512:(tr + 1) * 512], ps[tr][:],
                            bqk_t[:, jb:jb + 1])

            # ---------------- P2: v ----------------
            v_sb = [vres.tile([D, CP], BF16, name=f"v{tb}")
                    for tb in range(NTB)]
            with (
                tc.tile_pool(name="wvp", bufs=1) as wvp,
                tc.tile_pool(name="pv", bufs=4, space="PSUM") as pvp,
            ):
                wv_t = []
                for c in range(NC_CH):
                    t = wvp.tile([D, CP], BF16, name=f"wv{c}")
                    nc.sync.dma_start(t[:], wv[c * D:(c + 1) * D, :])
                    wv_t.append(t)
                NVR = CP // 512
                for tb in range(NTB):
                    ps = [pvp.tile([D, 512], F32, name="pv", tag="pv")
                          for _ in range(NVR)]
                    for c in range(NC_CH):
                        blk = xt[c][tb // 4][:, (tb % 4) * D:(tb % 4 + 1) * D]
                        for vr in range(NVR):
                            nc.tensor.matmul(
                                ps[vr][:], blk,
                                wv_t[c][:, vr * 512:(vr + 1) * 512],
                                start=(c == 0), stop=(c == NC_CH - 1))
                    for vr in range(NVR):
                        nc.vector.tensor_add(
                            v_sb[tb][:, vr * 512:(vr + 1) * 512], ps[vr][:],
                            bv_bc[:, vr * 512:(vr + 1) * 512])
            xtp_ctx.__exit__(None, None, None)

            # P4 weights: preload during P3 (rows already in arrival order)
            wpp_ctx = tc.tile_pool(name="wpp", bufs=1)
            wpp = wpp_ctx.__enter__()
            wp_t = []
            for k in range(NH_G):
                t = wpp.tile([D, CH], BF16, name=f"wp{k}")
                nc.sync.dma_start(t[:], wp[k * D:(k + 1) * D, :])
                wp_t.append(t)

            # ---------------- P3: attention + y exchange ----------------
            GSZ = 4
            ygs_ctx = tc.tile_pool(name="ygs", bufs=32)
            ygs = ygs_ctx.__enter__()
            yg_pre = [None] * NH_G
            with (
                tc.tile_pool(name="ylocal", bufs=2) as ylp,
                tc.tile_pool(name="attp", bufs=4) as attp,
                tc.tile_pool(name="ps_s", bufs=2, space="PSUM") as ps_s,
                tc.tile_pool(name="ps_o", bufs=2, space="PSUM") as ps_o,
                tc.tile_pool(name="ps_d", bufs=2, space="PSUM") as ps_d,
                tc.tile_pool(name="normp", bufs=4) as normp,
            ):
                for h in range(HPC):
                    qp = qks[2 * h]
                    kp = qks[2 * h + 1]
                    yt = ylp.tile([D, T], BF16, name="yt", tag="yt")
                    for ic in range(NIC):
                        ti0 = ic * TCH
                        ntk = (ti0 + TCH) // D
                        npair = ntk // 2
                        po = ps_o.tile([D, TCH], F32, name="po", tag="po")
                        pd = ps_d.tile([D, TCH], F32, name="pd", tag="pd")

                        def emit_s(pr):
                            # two key-tiles share one PSUM tile -> one exp
                            s_p = ps_s.tile([D, 2 * TCH], F32, name="sp",
                                            tag="sp")
                            for hf in range(2):
                                tk = 2 * pr + hf
                                nc.tensor.matmul(
                                    s_p[:, hf * TCH:(hf + 1) * TCH],
                                    kp[:, tk * D:(tk + 1) * D],
                                    qp[:, ti0:ti0 + TCH],
                                    start=True, stop=True)
                            att = attp.tile([D, 2 * TCH], BF16, name="att",
                                            tag="att")
                            nc.scalar.activation(att[:], s_p[:], AF.Exp,
                                                 bias=0.0, scale=scale)
                            # causal mask: x0/1 on idle GpSimd (keeps DVE free)
                            pdiag = pr - (npair - 2)
                            if pdiag >= 0:
                                nc.gpsimd.tensor_mul(
                                    att[:], att[:],
                                    mask_sb[:, pdiag * 2 * TCH:
                                            (pdiag + 1) * 2 * TCH])
                            return att

                        def emit_av(pr, att):
                            for hf in range(2):
                                tk = 2 * pr + hf
                                a = att[:, hf * TCH:(hf + 1) * TCH]
                                nc.tensor.matmul(
                                    po[:], v_sb[tk][:, h * D:(h + 1) * D], a,
                                    start=(tk == 0), stop=(tk == ntk - 1))
                                nc.tensor.matmul(
                                    pd[:], ones_sq[:], a,
                                    start=(tk == 0), stop=(tk == ntk - 1))

                        # software pipeline: S-pairs run 2 ahead of AV
                        atts = [emit_s(0)]
                        if npair > 1:
                            atts.append(emit_s(1))
                        for pr in range(npair):
                            if pr + 2 < npair:
                                atts.append(emit_s(pr + 2))
                            emit_av(pr, atts[pr])
                        rec = normp.tile([D, TCH], F32, name="rec", tag="rec")
                        nc.vector.reciprocal(rec[:], pd[:])
                        nc.vector.tensor_mul(yt[:, ti0:ti0 + TCH], po[:],
                                             rec[:])
                    nc.sync.dma_start(ag_in[h, :, :], yt[:])
                    if h % AG_CHUNK == AG_CHUNK - 1:
                        cchunk = h // AG_CHUNK
                        nc.gpsimd.collective_compute(
                            "AllGather",
                            mybir.AluOpType.bypass,
                            ins=[ag_in[cchunk * AG_CHUNK:
                                       (cchunk + 1) * AG_CHUNK, :, :]],
                            outs=[ag_out[cchunk, :, :, :, :]],
                            replica_groups=PAIRS,
                        )
                        # prefetch P4 group-0 y blocks for this chunk
                        for kk in range(4 * cchunk, 4 * cchunk + 4):
                            yg = ygs.tile([D, GSZ * D], BF16, name="yg",
                                          tag="yg")
                            nc.sync.dma_start(
                                yg[:], ag_out[kk // 4, (kk % 4) // 2, kk % 2,
                                              :, 0:GSZ * D])
                            yg_pre[kk] = yg
            # ---------------- P4: projection (all 16 global heads) -------
            # arrival order k -> ag_out[chunk=k//4, contrib=(k%4)//2, k%2]
            with (
                tc.tile_pool(name="pp", bufs=8, space="PSUM") as ppp,
                tc.tile_pool(name="post", bufs=3) as post,
            ):
                for g0 in range(0, NTB, GSZ):
                    if g0 == 0:
                        ygt = yg_pre
                    else:
                        ygt = []
                        for k in range(NH_G):
                            yg = ygs.tile([D, GSZ * D], BF16, name="yg",
                                          tag="yg")
                            nc.sync.dma_start(
                                yg[:], ag_out[k // 4, (k % 4) // 2, k % 2, :,
                                              g0 * D:(g0 + GSZ) * D])
                            ygt.append(yg)
                    pss = [[ppp.tile([D, 512], F32, name="pp", tag="pp")
                            for _ in range(NPR)] for _ in range(GSZ)]
                    for k in range(NH_G):
                        for ti in range(GSZ):
                            blk = ygt[k][:, ti * D:(ti + 1) * D]
                            for nr in range(NPR):
                                nc.tensor.matmul(
                                    pss[ti][nr][:], blk,
                                    wp_t[k][:, nr * 512:(nr + 1) * 512],
                                    start=(k == 0), stop=(k == NH_G - 1))
                    for ti in range(GSZ):
                        tb = g0 + ti
                        st = post.tile([D, CH], F32, name="pst", tag="pst")
                        for nr in range(NPR):
                            nc.vector.tensor_add(
                                st[:, nr * 512:(nr + 1) * 512],
                                pss[ti][nr][:],
                                bp_bc[:, nr * 512:(nr + 1) * 512])
                        nc.sync.dma_start(
                            out_ext[tb * D:(tb + 1) * D, :], st[:])
            ygs_ctx.__exit__(None, None, None)
            wpp_ctx.__exit__(None, None, None)
            vres_ctx.__exit__(None, None, None)
            qkp_ctx.__exit__(None, None, None)
    nc.finalize()
    return nc


def _prep_inputs(cfg: Cfg, x, w_attn, b_attn, w_proj, b_proj):
    """Host-side shard/cast. Returns in_maps (list of dicts per core)."""
    T, C, CP, HPC, CH = cfg.T, cfg.C, cfg.CP, cfg.HPC, cfg.CH
    bf = ml_dtypes.bfloat16
    wq = w_attn[:, 0:C]
    wk = w_attn[:, C:2 * C]
    wvf = w_attn[:, 2 * C:3 * C]
    bq, bk, bvf = b_attn[0:C], b_attn[C:2 * C], b_attn[2 * C:3 * C]

    masks = np.zeros((D, 4 * cfg.TCH), dtype=bf)
    f = np.arange(cfg.TCH)[None, :]
    p = np.arange(D)[:, None]
    for k in range(4):
        keep = (f - p >= 128 * k)
        masks[:, k * cfg.TCH:(k + 1) * cfg.TCH] = np.where(
            keep, 1.0, 0.0).astype(bf)

    # P4 contraction (arrival) order: for AG chunk c, contributor m, head j
    # -> global head 8*m + 2*c + j
    arrival = [8 * m + AG_CHUNK * c + j
               for c in range(HPC // AG_CHUNK)
               for m in range(2)
               for j in range(AG_CHUNK)]

    in_maps = []
    for core in range(N_CORES):
        b = core // 2
        g = core % 2
        h0 = g * HPC * D            # first col of this head group
        sl = slice(h0, h0 + CP)
        xTc = np.ascontiguousarray(x[b].T).astype(bf)
        wqk_cols = []
        for h in range(HPC):
            hs = slice(h0 + h * D, h0 + (h + 1) * D)
            wqk_cols.append(wq[:, hs])
            wqk_cols.append(wk[:, hs])
        wqk_c = np.concatenate(wqk_cols, axis=1).astype(bf)
        wv_c = wvf[:, sl].astype(bf)
        # full-row proj weights, rows in arrival order, columns = core half
        wp_rows = [w_proj[gh * D:(gh + 1) * D, g * CH:(g + 1) * CH]
                   for gh in arrival]
        wp_c = np.concatenate(wp_rows, axis=0).astype(bf)
        bqk_cols = []
        for h in range(HPC):
            hs = slice(h0 + h * D, h0 + (h + 1) * D)
            bqk_cols.append(bq[hs])
            bqk_cols.append(bk[hs])
        bqk_c = np.ascontiguousarray(np.stack(bqk_cols, axis=1)).astype(np.float32)
        in_maps.append({
            "xT": xTc,
            "wqk": wqk_c,
            "wv": wv_c,
            "wp": wp_c,
            "bqk": bqk_c,
            "bv": bvf[sl].reshape(1, CP).astype(bf),
            "bp": b_proj[g * CH:(g + 1) * CH].reshape(1, CH).astype(bf),
            "masks": masks,
        })
    return in_maps


_CFG = Cfg()


def kernel(x, w_attn, b_attn, w_proj, b_proj, _trace=False, _cfg=None):
    from concourse.bass_utils import run_bass_kernel_spmd
    cfg = _cfg or _CFG
    x = np.asarray(x, dtype=np.float32)
    w_attn = np.asarray(w_attn, dtype=np.float32)
    b_attn = np.asarray(b_attn, dtype=np.float32)
    w_proj = np.asarray(w_proj, dtype=np.float32)
    b_proj = np.asarray(b_proj, dtype=np.float32)

    in_maps = _prep_inputs(cfg, x, w_attn, b_attn, w_proj, b_proj)
    nc = build_kernel(cfg)
    res = run_bass_kernel_spmd(nc, in_maps, list(range(N_CORES)), trace=_trace)
    outs = []
    for b in range(cfg.B):
        left = res.results[2 * b]["out"]
        right = res.results[2 * b + 1]["out"]
        outs.append(np.concatenate([left, right], axis=1))
    full = np.stack(outs, axis=0).astype(np.float32)
    if _trace:
        kernel.last_exec_time_ns = res.exec_time_ns
        kernel.last_mean_exec_time_ns = res.mean_exec_time_ns
        kernel.last_scope_times = res.per_core_scope_times
    return full


# revision 27
# speedup vs baseline: 1.1245x; 1.0376x over previous
"""Trainium2 Bass kernel for causal self-attention (B=4, T=2048, C=2048, H=16).

Sharding: 8 cores = DP4 (batch) x TP2 (8 heads each). Each core:
  P1  qk' = (x @ Wqk)^T computed in [j, t] layout, kept resident in SBUF
  P2  v   = x @ Wv in natural [t, j] layout (stationary = xT slab blocks)
  P3  per head: s_T = k'^T q' -> exp -> causal mask -> av + ones-den
      -> y'_h = o' * recip(den); y'_h staged to DRAM; every 2 heads an
      AllGather over the core pair exchanges y so both cores hold all 16
      global heads' y (overlapped with remaining attention compute).
  P4  out[t, n_half] = sum over all 16 global heads y'^T @ Wp + bias,
      written directly to the external output (each core owns half the
      output columns; no post-hoc reduction needed). y blocks streamed
      back from the AllGather DRAM buffer.

All matmuls bf16 with fp32 PSUM accumulation; softmax in fp32 on ACT/DVE.
Host side: shard/cast/transpose inputs, concat output column halves.
"""
import math
import numpy as np
import ml_dtypes

import concourse.bass as bass
import concourse.bacc as bacc
import concourse.mybir as mybir
import concourse.tile as tile

F32 = mybir.dt.float32
BF16 = mybir.dt.bfloat16
AF = mybir.ActivationFunctionType

D = 128          # head dim (fixed: partition size)
N_CORES = 8
PAIRS = [[0, 1], [2, 3], [4, 5], [6, 7]]
AG_CHUNK = 2     # heads per AllGather


class Cfg:
    def __init__(self, T=2048, H_TOT=16, HPC=8, B=4):
        self.T = T                    # sequence length
        self.H_TOT = H_TOT            # total heads
        self.HPC = HPC                # heads per core
        self.B = B
        self.C = H_TOT * D            # model dim
        self.CP = HPC * D             # per-core head cols
        self.CH = self.C // 2         # per-core output cols
        self.TCH = 512                # ti chunk width
        assert T % self.TCH == 0 and T % D == 0


def build_kernel(cfg: Cfg):
    T, C, CP, HPC, TCH, CH = cfg.T, cfg.C, cfg.CP, cfg.HPC, cfg.TCH, cfg.CH
    NC_CH = C // D                # c-chunks (contraction)
    NJB = 2 * HPC                 # qk' j-blocks (q heads then k heads)
    NTB = T // D                  # t-blocks
    NIC = T // TCH                # ti chunks
    NTR = T // 512                # t-ranges (slab columns)
    NPR = CH // 512               # proj n-ranges
    NAG = HPC // AG_CHUNK         # AllGather chunks
    NH_G = 2 * HPC                # global heads in P4 contraction
    scale = 1.0 / math.sqrt(D)

    nc = bacc.Bacc()
    xT = nc.declare_dram_parameter("xT", [C, T], BF16, isOutput=False)
    wqk = nc.declare_dram_parameter("wqk", [C, 2 * CP], BF16, isOutput=False)
    wv = nc.declare_dram_parameter("wv", [C, CP], BF16, isOutput=False)
    wp = nc.declare_dram_parameter("wp", [C, CH], BF16, isOutput=False)
    bqk = nc.declare_dram_parameter("bqk", [D, NJB], F32, isOutput=False)
    bv = nc.declare_dram_parameter("bv", [1, CP], BF16, isOutput=False)
    bp = nc.declare_dram_parameter("bp", [1, CH], BF16, isOutput=False)
    masks = nc.declare_dram_parameter("masks", [D, 4 * TCH], BF16, isOutput=False)
    out_ext = nc.declare_dram_parameter("out", [T, CH], F32, isOutput=True)

    ag_in = nc.dram_tensor("ag_in", [HPC, D, T], BF16)
    ag_out = nc.dram_tensor("ag_out", [NAG, 2, AG_CHUNK, D, T], BF16)

    with tile.TileContext(nc) as tc:
        with tc.tile_pool(name="const", bufs=1) as constp:
            bqk_t = constp.tile([D, NJB], F32, name="bqk_t")
            nc.sync.dma_start(bqk_t[:], bqk[:, :])
            bv_t = constp.tile([1, CP], BF16, name="bv_t")
            nc.sync.dma_start(bv_t[:], bv[:, :])
            bp_t = constp.tile([1, CH], BF16, name="bp_t")
            nc.sync.dma_start(bp_t[:], bp[:, :])
            mask_sb = constp.tile([D, 4 * TCH], BF16, name="mask_sb")
            nc.sync.dma_start(mask_sb[:], masks[:, :])
            ones_sq = constp.tile([D, D], BF16, name="ones_sq")
            nc.vector.memset(ones_sq[:], 1.0)
            ones_row = constp.tile([1, D], BF16, name="ones_row")
            nc.vector.memset(ones_row[:], 1.0)

            # PE warmup: ~3.5us of dummy matmuls on memset tiles while the
            # first input DMAs land, so P1 starts at the warm 2.4 GHz clock
            warm_in = constp.tile([D, 512], BF16, name="warm_in")
            nc.vector.memset(warm_in[:], 0.0)

            # broadcast bias rows -> [D, .] tiles (ones-matmul, one time)
            bv_bc = constp.tile([D, CP], BF16, name="bv_bc")
            bp_bc = constp.tile([D, CH], BF16, name="bp_bc")
            with tc.tile_pool(name="pbias", bufs=4, space="PSUM") as pbias:
                for _ in range(18):
                    pw = pbias.tile([D, 512], F32, name="pw", tag="pb")
                    nc.tensor.matmul(pw[:], ones_sq[:], warm_in[:],
                                     start=True, stop=True)
                for vr in range(CP // 512):
                    psb = pbias.tile([D, 512], F32, name="psbv", tag="pb")
                    nc.tensor.matmul(psb[:], ones_row[:],
                                     bv_t[:, vr * 512:(vr + 1) * 512],
                                     start=True, stop=True)
                    nc.vector.tensor_copy(bv_bc[:, vr * 512:(vr + 1) * 512],
                                          psb[:])
                for nr in range(NPR):
                    psb = pbias.tile([D, 512], F32, name="psbp", tag="pb")
                    nc.tensor.matmul(psb[:], ones_row[:],
                                     bp_t[:, nr * 512:(nr + 1) * 512],
                                     start=True, stop=True)
                    nc.vector.tensor_copy(bp_bc[:, nr * 512:(nr + 1) * 512],
                                          psb[:])

            # manual-lifetime pools (reserve-at-push, LIFO release):
            # qkp/vres live to P3 end, xtp to P2 end, wpp pushed after xtp
            # pops and lives to P4 end.
            qkp_ctx = tc.tile_pool(name="qkp", bufs=1)
            qkp = qkp_ctx.__enter__()
            vres_ctx = tc.tile_pool(name="vres", bufs=1)
            vres = vres_ctx.__enter__()
            xtp_ctx = tc.tile_pool(name="xtp", bufs=1)
            xtp = xtp_ctx.__enter__()
            xt = [[None] * NTR for _ in range(NC_CH)]

            # resident qk' tiles, [D, T] per j-block; freed after P3
            qks = [qkp.tile([D, T], BF16, name=f"qks{jb}") for jb in range(NJB)]

            # ---------------- P1: qk' ----------------
            with (
                tc.tile_pool(name="wqkp", bufs=32) as wqkp,
                tc.tile_pool(name="pq", bufs=8, space="PSUM") as pqp,
            ):
                # quad 0 weights interleaved with x slabs so jb0 starts fast
                wq_cur = []
                for c in range(NC_CH):
                    wt = wqkp.tile([D, 512], BF16, name="wq", tag="wq")
                    nc.sync.dma_start(wt[:], wqk[c * D:(c + 1) * D, 0:512])
                    wq_cur.append(wt)
                    for tr in range(NTR):
                        st = xtp.tile([D, 512], BF16, name=f"xt{c}_{tr}")
                        nc.sync.dma_start(
                            st[:], xT[c * D:(c + 1) * D, tr * 512:(tr + 1) * 512])
                        xt[c][tr] = st
                for jb in range(NJB):
                    if jb % 4 == 0 and jb > 0:
                        wq_cur = []
                        for c in range(NC_CH):
                            wt = wqkp.tile([D, 512], BF16, name="wq", tag="wq")
                            nc.sync.dma_start(
                                wt[:], wqk[c * D:(c + 1) * D,
                                           jb * D:(jb + 4) * D])
                            wq_cur.append(wt)
                    jo = (jb % 4) * D
                    ps = [pqp.tile([D, 512], F32, name="pq", tag="pq")
                          for _ in range(NTR)]
                    for c in range(NC_CH):
                        for tr in range(NTR):
                            nc.tensor.matmul(
                                ps[tr][:], wq_cur[c][:, jo:jo + D],
                                xt[c][tr][:],
                                start=(c == 0), stop=(c == NC_CH - 1))
                    for tr in range(NTR):
                        nc.vector.tensor_scalar_add(
                            qks[jb][:, tr * 512:(tr + 1) * 512], ps[tr][:],
                            bqk_t[:, jb:jb + 1])

            # ---------------- P2: v ----------------
            v_sb = [vres.tile([D, CP], BF16, name=f"v{tb}")
                    for tb in range(NTB)]
            with (
                tc.tile_pool(name="wvp", bufs=1) as wvp,
                tc.tile_pool(name="pv", bufs=4, space="PSUM") as pvp,
            ):
                wv_t = []
                for c in range(NC_CH):
                    t = wvp.tile([D, CP], BF16, name=f"wv{c}")
                    nc.sync.dma_start(t[:], wv[c * D:(c + 1) * D, :])
                    wv_t.append(t)
                NVR = CP // 512
                for tb in range(NTB):
                    ps = [pvp.tile([D, 512], F32, name="pv", tag="pv")
                          for _ in range(NVR)]
                    for c in range(NC_CH):
                        blk = xt[c][tb // 4][:, (tb % 4) * D:(tb % 4 + 1) * D]
                        for vr in range(NVR):
                            nc.tensor.matmul(
                                ps[vr][:], blk,
                                wv_t[c][:, vr * 512:(vr + 1) * 512],
                                start=(c == 0), stop=(c == NC_CH - 1))
                    for vr in range(NVR):
                        nc.vector.tensor_add(
                            v_sb[tb][:, vr * 512:(vr + 1) * 512], ps[vr][:],
                            bv_bc[:, vr * 512:(vr + 1) * 512])
            xtp_ctx.__exit__(None, None, None)

            # P4 weights: preload during P3 (rows already in arrival order)
            wpp_ctx = tc.tile_pool(name="wpp", bufs=1)
            wpp = wpp_ctx.__enter__()
            wp_t = []
            for k in range(NH_G):
                t = wpp.tile([D, CH], BF16, name=f"wp{k}")
                nc.sync.dma_start(t[:], wp[k * D:(k + 1) * D, :])
                wp_t.append(t)

            # ---------------- P3: attention + y exchange ----------------
            GSZ = 4
            ygs_ctx = tc.tile_pool(name="ygs", bufs=32)
            ygs = ygs_ctx.__enter__()
            yg_pre = [None] * NH_G
            with (
                tc.tile_pool(name="ylocal", bufs=2) as ylp,
                tc.tile_pool(name="attp", bufs=4) as attp,
                tc.tile_pool(name="ps_s", bufs=2, space="PSUM") as ps_s,
                tc.tile_pool(name="ps_o", bufs=2, space="PSUM") as ps_o,
                tc.tile_pool(name="ps_d", bufs=2, space="PSUM") as ps_d,
                tc.tile_pool(name="normp", bufs=4) as normp,
            ):
                for h in range(HPC):
                    qp = qks[2 * h]
                    kp = qks[2 * h + 1]
                    yt = ylp.tile([D, T], BF16, name="yt", tag="yt")
                    for ic in range(NIC - 1, -1, -1):
                        ti0 = ic * TCH
                        ntk = (ti0 + TCH) // D
                        npair = ntk // 2
                        po = ps_o.tile([D, TCH], F32, name="po", tag="po")
                        pd = ps_d.tile([D, TCH], F32, name="pd", tag="pd")

                        def emit_s(pr):
                            # two key-tiles share one PSUM tile -> one exp
                            s_p = ps_s.tile([D, 2 * TCH], F32, name="sp",
                                            tag="sp")
                            for hf in range(2):
                                tk = 2 * pr + hf
                                nc.tensor.matmul(
                                    s_p[:, hf * TCH:(hf + 1) * TCH],
                                    kp[:, tk * D:(tk + 1) * D],
                                    qp[:, ti0:ti0 + TCH],
                                    start=True, stop=True)
                            att = attp.tile([D, 2 * TCH], BF16, name="att",
                                            tag="att")
                            nc.scalar.activation(att[:], s_p[:], AF.Exp,
                                                 bias=0.0, scale=scale)
                            # causal mask: multiply by 0/1 (bf16 SBUF, 2x DVE)
                            pdiag = pr - (npair - 2)
                            if pdiag >= 0:
                                nc.vector.tensor_mul(
                                    att[:], att[:],
                                    mask_sb[:, pdiag * 2 * TCH:
                                            (pdiag + 1) * 2 * TCH])
                            return att

                        def emit_av(pr, att):
                            # po first (feeds the y critical path), den after
                            for hf in range(2):
                                tk = 2 * pr + hf
                                a = att[:, hf * TCH:(hf + 1) * TCH]
                                nc.tensor.matmul(
                                    po[:], v_sb[tk][:, h * D:(h + 1) * D], a,
                                    start=(tk == 0), stop=(tk == ntk - 1))
                            for hf in range(2):
                                tk = 2 * pr + hf
                                a = att[:, hf * TCH:(hf + 1) * TCH]
                                nc.tensor.matmul(
                                    pd[:], ones_sq[:], a,
                                    start=(tk == 0), stop=(tk == ntk - 1))

                        # software pipeline: S-pairs run 2 ahead of AV
                        atts = [emit_s(0)]
                        if npair > 1:
                            atts.append(emit_s(1))
                        for pr in range(npair):
                            if pr + 2 < npair:
                                atts.append(emit_s(pr + 2))
                            emit_av(pr, atts[pr])
                        rec = normp.tile([D, TCH], F32, name="rec", tag="rec")
                        nc.vector.reciprocal(rec[:], pd[:])
                        nc.vector.tensor_mul(yt[:, ti0:ti0 + TCH], po[:],
                                             rec[:])
                    nc.sync.dma_start(ag_in[h, :, :], yt[:])
                    if h % AG_CHUNK == AG_CHUNK - 1:
                        cchunk = h // AG_CHUNK
                        nc.gpsimd.collective_compute(
                            "AllGather",
                            mybir.AluOpType.bypass,
                            ins=[ag_in[cchunk * AG_CHUNK:
                                       (cchunk + 1) * AG_CHUNK, :, :]],
                            outs=[ag_out[cchunk, :, :, :, :]],
                            replica_groups=PAIRS,
                        )
                        # prefetch P4 group-0 y blocks for this chunk
                        for kk in range(4 * cchunk, 4 * cchunk + 4):
                            yg = ygs.tile([D, GSZ * D], BF16, name="yg",
                                          tag="yg")
                            nc.sync.dma_start(
                                yg[:], ag_out[kk // 4, (kk % 4) // 2, kk % 2,
                                              :, 0:GSZ * D])
                            yg_pre[kk] = yg
            # ---------------- P4: projection (all 16 global heads) -------
            # arrival order k -> ag_out[chunk=k//4, contrib=(k%4)//2, k%2]
            with (
                tc.tile_pool(name="pp", bufs=8, space="PSUM") as ppp,
                tc.tile_pool(name="post", bufs=3) as post,
            ):
                for g0 in range(0, NTB, GSZ):
                    if g0 == 0:
                        ygt = yg_pre
                    else:
                        ygt = []
                        for k in range(NH_G):
                            yg = ygs.tile([D, GSZ * D], BF16, name="yg",
                                          tag="yg")
                            nc.sync.dma_start(
                                yg[:], ag_out[k // 4, (k % 4) // 2, k % 2, :,
                                              g0 * D:(g0 + GSZ) * D])
                            ygt.append(yg)
                    pss = [[ppp.tile([D, 512], F32, name="pp", tag="pp")
                            for _ in range(NPR)] for _ in range(GSZ)]
                    for k in range(NH_G):
                        for ti in range(GSZ):
                            blk = ygt[k][:, ti * D:(ti + 1) * D]
                            for nr in range(NPR):
                                nc.tensor.matmul(
                                    pss[ti][nr][:], blk,
                                    wp_t[k][:, nr * 512:(nr + 1) * 512],
                                    start=(k == 0), stop=(k == NH_G - 1))
                    for ti in range(GSZ):
                        tb = g0 + ti
                        st = post.tile([D, CH], F32, name="pst", tag="pst")
                        for nr in range(NPR):
                            nc.vector.tensor_add(
                                st[:, nr * 512:(nr + 1) * 512],
                                pss[ti][nr][:],
                                bp_bc[:, nr * 512:(nr + 1) * 512])
                        nc.sync.dma_start(
                            out_ext[tb * D:(tb + 1) * D, :], st[:])
            ygs_ctx.__exit__(None, None, None)
            wpp_ctx.__exit__(None, None, None)
            vres_ctx.__exit__(None, None, None)
            qkp_ctx.__exit__(None, None, None)
    nc.finalize()
    return nc


def _prep_inputs(cfg: Cfg, x, w_attn, b_attn, w_proj, b_proj):
    """Host-side shard/cast. Returns in_maps (list of dicts per core)."""
    T, C, CP, HPC, CH = cfg.T, cfg.C, cfg.CP, cfg.HPC, cfg.CH
    bf = ml_dtypes.bfloat16
    wq = w_attn[:, 0:C]
    wk = w_attn[:, C:2 * C]
    wvf = w_attn[:, 2 * C:3 * C]
    bq, bk, bvf = b_attn[0:C], b_attn[C:2 * C], b_attn[2 * C:3 * C]

    masks = np.zeros((D, 4 * cfg.TCH), dtype=bf)
    f = np.arange(cfg.TCH)[None, :]
    p = np.arange(D)[:, None]
    for k in range(4):
        keep = (f - p >= 128 * k)
        masks[:, k * cfg.TCH:(k + 1) * cfg.TCH] = np.where(
            keep, 1.0, 0.0).astype(bf)

    # P4 contraction (arrival) order: for AG chunk c, contributor m, head j
    # -> global head 8*m + 2*c + j
    arrival = [8 * m + AG_CHUNK * c + j
               for c in range(HPC // AG_CHUNK)
               for m in range(2)
               for j in range(AG_CHUNK)]

    in_maps = []
    for core in range(N_CORES):
        b = core // 2
        g = core % 2
        h0 = g * HPC * D            # first col of this head group
        sl = slice(h0, h0 + CP)
        xTc = np.ascontiguousarray(x[b].T).astype(bf)
        wqk_cols = []
        for h in range(HPC):
            hs = slice(h0 + h * D, h0 + (h + 1) * D)
            wqk_cols.append(wq[:, hs])
            wqk_cols.append(wk[:, hs])
        wqk_c = np.concatenate(wqk_cols, axis=1).astype(bf)
        wv_c = wvf[:, sl].astype(bf)
        # full-row proj weights, rows in arrival order, columns = core half
        wp_rows = [w_proj[gh * D:(gh + 1) * D, g * CH:(g + 1) * CH]
                   for gh in arrival]
        wp_c = np.concatenate(wp_rows, axis=0).astype(bf)
        bqk_cols = []
        for h in range(HPC):
            hs = slice(h0 + h * D, h0 + (h + 1) * D)
            bqk_cols.append(bq[hs])
            bqk_cols.append(bk[hs])
        bqk_c = np.ascontiguousarray(np.stack(bqk_cols, axis=1)).astype(np.float32)
        in_maps.append({
            "xT": xTc,
            "wqk": wqk_c,
            "wv": wv_c,
            "wp": wp_c,
            "bqk": bqk_c,
            "bv": bvf[sl].reshape(1, CP).astype(bf),
            "bp": b_proj[g * CH:(g + 1) * CH].reshape(1, CH).astype(bf),
            "masks": masks,
        })
    return in_maps


_CFG = Cfg()


def kernel(x, w_attn, b_attn, w_proj, b_proj, _trace=False, _cfg=None):
    from concourse.bass_utils import run_bass_kernel_spmd
    cfg = _cfg or _CFG
    x = np.asarray(x, dtype=np.float32)
    w_attn = np.asarray(w_attn, dtype=np.float32)
    b_attn = np.asarray(b_attn, dtype=np.float32)
    w_proj = np.asarray(w_proj, dtype=np.float32)
    b_proj = np.asarray(b_proj, dtype=np.float32)

    in_maps = _prep_inputs(cfg, x, w_attn, b_attn, w_proj, b_proj)
    nc = build_kernel(cfg)
    res = run_bass_kernel_spmd(nc, in_maps, list(range(N_CORES)), trace=_trace)
    outs = []
    for b in range(cfg.B):
        left = res.results[2 * b]["out"]
        right = res.results[2 * b + 1]["out"]
        outs.append(np.concatenate([left, right], axis=1))
    full = np.stack(outs, axis=0).astype(np.float32)
    if _trace:
        kernel.last_exec_time_ns = res.exec_time_ns
        kernel.last_mean_exec_time_ns = res.mean_exec_time_ns
        kernel.last_scope_times = res.per_core_scope_times
    return full
